# revision 7
# baseline (speedup 1.0000x reference)
import sys
sys.path.insert(0, '/opt/trn_rl_repo')
import numpy as np
import concourse.bass as bass
import concourse.bacc as bacc
import concourse.tile as tile
import concourse.mybir as mybir
from concourse.bass_utils import run_bass_kernel_spmd

# ---- problem constants (hardcoded; kernel.py must be self-contained) ----
C3_TABLE = [(0, 1, 2), (1, 2, 3), (2, 3, 4), (3, 4, 5), (0, 4, 5), (0, 1, 5),
            (0, 1, 2, 3), (1, 2, 3, 4), (2, 3, 4, 5), (0, 3, 4, 5), (0, 1, 4, 5),
            (0, 1, 2, 5), (0, 1, 3, 4), (1, 2, 4, 5), (0, 2, 3, 5),
            (0, 1, 2, 3, 4, 5)]
A = 1.7159
S = 2.0 / 3.0

B, C, H, W = 256, 6, 142, 142
KH = KW = 5
OC = 16
OH, OW = H - 4, W - 4          # 138, 138
NCORES = 8
B_LOC = B // NCORES            # 32 images per core

T = 8                          # oh rows per block (j dim in M)
HH = T + KH - 1                # 12 input rows per block
K = C * HH                     # 72 contraction partitions
BPER = 2                       # images per matmul stream
N = BPER * W                   # 284 free elems in X tile
NS = BPER * OW                 # 276 streamed columns per matmul
NBLK = (OH + T - 1) // T       # 18 blocks (last one overlaps)
NPAIR = B_LOC // BPER          # 16 image pairs

_cache = {}


def _build():
    if 'nc' in _cache:
        return _cache['nc']
    f32 = mybir.dt.float32
    f32r = mybir.dt.float32r
    nc = bacc.Bacc("TRN2", target_bir_lowering=False, debug=False,
                   num_devices=NCORES)
    x_d = nc.dram_tensor("x", [B_LOC, C, H, W], f32, kind="ExternalInput").ap()
    w_d = nc.dram_tensor("w", [K, KW, 128], f32, kind="ExternalInput").ap()
    b_d = nc.dram_tensor("b", [128, 1], f32, kind="ExternalInput").ap()
    y_d = nc.dram_tensor("y", [B_LOC, OC, OH, OW], f32, kind="ExternalOutput").ap()

    with tile.TileContext(nc) as tc:
        with tc.tile_pool(name="wpool", bufs=1) as wpool, \
             tc.tile_pool(name="xpool", bufs=4) as xpool, \
             tc.tile_pool(name="opool", bufs=4) as opool, \
             tc.tile_pool(name="pspool", bufs=4, space="PSUM") as pspool:
            w_sb = wpool.tile([K, KW * 128], f32r)
            nc.sync.dma_start(w_sb[:], w_d[:].bitcast(f32r).rearrange("k f m -> k (f m)"))
            b_sb = wpool.tile([128, 1], f32)
            nc.sync.dma_start(b_sb[:], b_d[:])

            for pair in range(NPAIR):
                b0 = pair * BPER
                for blk in range(NBLK):
                    oh0 = min(blk * T, OH - T)   # last block overlaps (130)
                    xt = xpool.tile([K, N], f32r)
                    for c in range(C):
                        # SBUF [12, 284] <- DRAM [12, 2, 142]
                        src = x_d[b0:b0 + BPER, c, oh0:oh0 + HH, :]
                        src = src.transpose([1, 0, 2])
                        nc.sync.dma_start(xt[c * HH:(c + 1) * HH, :],
                                          src.bitcast(f32r))

                    ps = pspool.tile([128, NS], f32)
                    xv = xt[:].rearrange("k (b w) -> k b w", b=BPER)
                    for kw in range(KW):
                        nc.tensor.matmul(
                            ps[:],
                            w_sb[:, kw * 128:(kw + 1) * 128],
                            xv[:, :, kw:kw + OW],
                            start=(kw == 0), stop=(kw == KW - 1),
                        )

                    t_sb = opool.tile([128, NS], f32, tag="tanh")
                    nc.scalar.activation(t_sb[:], ps[:],
                                         mybir.ActivationFunctionType.Tanh,
                                         bias=b_sb[:], scale=S)
                    o_sb = opool.tile([128, NS], f32, tag="out")
                    nc.vector.tensor_scalar_mul(o_sb[:], t_sb[:], A)

                    # full write each block; last block overlaps block 16
                    # with identical values (WAW on same data is benign)
                    for b2 in range(BPER):
                        odst = y_d[b0 + b2, :, oh0:oh0 + T, :]   # [16,8,138]
                        nc.sync.dma_start(odst, o_sb[:, b2 * OW:(b2 + 1) * OW])
    nc.compile()
    _cache['nc'] = nc
    return nc


def _prep_weights(w3, b3, w4, b4, w6, b6):
    Wd = np.zeros((OC, C, KH, KW), np.float32)
    bias = np.zeros((OC,), np.float32)
    for i, idx in enumerate(C3_TABLE[:6]):
        Wd[i, list(idx)] = w3[i]
        bias[i] = b3[i]
    for i, idx in enumerate(C3_TABLE[6:15]):
        Wd[6 + i, list(idx)] = w4[i]
        bias[6 + i] = b4[i]
    Wd[15, list(C3_TABLE[15])] = w6[0]
    bias[15] = b6[0]

    # M is oc-major: m = oc*T + j  (matches out DRAM order [oc, oh, ow])
    wk = np.zeros((K, KW, 128), np.float32)
    for c in range(C):
        for hh in range(HH):
            for j in range(T):
                kh = hh - j
                if 0 <= kh < KH:
                    for oc in range(OC):
                        wk[c * HH + hh, :, oc * T + j] = Wd[oc, c, kh, :]
    bvec = (S * bias[np.arange(128) // T]).reshape(128, 1).astype(np.float32)
    return wk, bvec


def kernel(x, w3, b3, w4, b4, w6, b6):
    nc = _build()
    wk, bvec = _prep_weights(w3, b3, w4, b4, w6, b6)
    x = np.ascontiguousarray(x, dtype=np.float32)
    in_maps = [{"x": x[i * B_LOC:(i + 1) * B_LOC], "w": wk, "b": bvec}
               for i in range(NCORES)]
    res = run_bass_kernel_spmd(nc, in_maps, list(range(NCORES)))
    out = np.concatenate([res.results[i]["y"] for i in range(NCORES)], axis=0)
    return out


# revision 9
# speedup vs baseline: 5.4973x; 5.4973x over previous
import sys
sys.path.insert(0, '/opt/trn_rl_repo')
import numpy as np
import concourse.bass as bass
import concourse.bacc as bacc
import concourse.tile as tile
import concourse.mybir as mybir
from concourse.bass_utils import run_bass_kernel_spmd

# ---- problem constants (hardcoded; kernel.py must be self-contained) ----
C3_TABLE = [(0, 1, 2), (1, 2, 3), (2, 3, 4), (3, 4, 5), (0, 4, 5), (0, 1, 5),
            (0, 1, 2, 3), (1, 2, 3, 4), (2, 3, 4, 5), (0, 3, 4, 5), (0, 1, 4, 5),
            (0, 1, 2, 5), (0, 1, 3, 4), (1, 2, 4, 5), (0, 2, 3, 5),
            (0, 1, 2, 3, 4, 5)]
A = 1.7159
S = 2.0 / 3.0

B, C, H, W = 256, 6, 142, 142
KH = KW = 5
OC = 16
OH, OW = H - 4, W - 4          # 138, 138
NCORES = 8
B_LOC = B // NCORES            # 32 images per core

T = 8                          # oh rows per block (j dim in M)
HH = T + KH - 1                # 12 input rows per block
K = C * HH                     # 72 contraction partitions
BPER = 2                       # images per matmul stream
NS = BPER * OW                 # 276 streamed columns per matmul
NBLK = (OH + T - 1) // T       # 18 blocks (last one overlaps)
NPAIR = B_LOC // BPER          # 16 image pairs
PGP = 8                        # pairs per group (16 images)
NPG = NPAIR // PGP             # 2 groups
GI = PGP * BPER                # 16 images per group
XF = GI * W                    # 2272 free elems in X tile
SF = PGP * NS                  # 2208 free elems in staging tile

_cache = {}


def _build():
    if 'nc' in _cache:
        return _cache['nc']
    f32 = mybir.dt.float32
    f32r = mybir.dt.float32r
    nc = bacc.Bacc("TRN2", target_bir_lowering=False, debug=False,
                   num_devices=NCORES)
    x_d = nc.dram_tensor("x", [NBLK, K, B_LOC, W], f32, kind="ExternalInput").ap()
    w_d = nc.dram_tensor("w", [K, KW, 128], f32, kind="ExternalInput").ap()
    b_d = nc.dram_tensor("b", [128, 1], f32, kind="ExternalInput").ap()
    y_d = nc.dram_tensor("y", [NBLK, NPG, 128, SF], f32, kind="ExternalOutput").ap()

    with tile.TileContext(nc) as tc:
        with tc.tile_pool(name="wpool", bufs=1) as wpool, \
             tc.tile_pool(name="xpool", bufs=3) as xpool, \
             tc.tile_pool(name="tpool", bufs=4) as tpool, \
             tc.tile_pool(name="spool", bufs=3) as spool, \
             tc.tile_pool(name="pspool", bufs=8, space="PSUM") as pspool:
            w_sb = wpool.tile([K, KW * 128], f32r)
            nc.sync.dma_start(w_sb[:], w_d[:].bitcast(f32r).rearrange("k f m -> k (f m)"))
            b_sb = wpool.tile([128, 1], f32)
            nc.sync.dma_start(b_sb[:], b_d[:])

            for blk in range(NBLK):
                for pg in range(NPG):
                    xt = xpool.tile([K, XF], f32r)
                    src = x_d[blk, :, pg * GI:(pg + 1) * GI, :]   # [72,16,142]
                    src = src.rearrange("k i w -> k (i w)")
                    nc.sync.dma_start(xt[:], src.bitcast(f32r))

                    stage = spool.tile([128, SF], f32)
                    xv = xt[:].rearrange("k (i w) -> k i w", i=GI)
                    for pair in range(PGP):
                        ps = pspool.tile([128, NS], f32)
                        rv = xv[:, pair * BPER:(pair + 1) * BPER, :]
                        for kw in range(KW):
                            nc.tensor.matmul(
                                ps[:],
                                w_sb[:, kw * 128:(kw + 1) * 128],
                                rv[:, :, kw:kw + OW],
                                start=(kw == 0), stop=(kw == KW - 1),
                            )
                        t_sb = tpool.tile([128, NS], f32)
                        nc.scalar.activation(t_sb[:], ps[:],
                                             mybir.ActivationFunctionType.Tanh,
                                             bias=b_sb[:], scale=S)
                        nc.vector.tensor_scalar_mul(
                            stage[:, pair * NS:(pair + 1) * NS], t_sb[:], A)
                    nc.sync.dma_start(y_d[blk, pg], stage[:])
    nc.compile()
    _cache['nc'] = nc
    return nc


def _prep_weights(w3, b3, w4, b4, w6, b6):
    Wd = np.zeros((OC, C, KH, KW), np.float32)
    bias = np.zeros((OC,), np.float32)
    for i, idx in enumerate(C3_TABLE[:6]):
        Wd[i, list(idx)] = w3[i]
        bias[i] = b3[i]
    for i, idx in enumerate(C3_TABLE[6:15]):
        Wd[6 + i, list(idx)] = w4[i]
        bias[6 + i] = b4[i]
    Wd[15, list(C3_TABLE[15])] = w6[0]
    bias[15] = b6[0]

    # M is oc-major: m = oc*T + j
    wk = np.zeros((K, KW, 128), np.float32)
    for c in range(C):
        for hh in range(HH):
            for j in range(T):
                kh = hh - j
                if 0 <= kh < KH:
                    for oc in range(OC):
                        wk[c * HH + hh, :, oc * T + j] = Wd[oc, c, kh, :]
    bvec = (S * bias[np.arange(128) // T]).reshape(128, 1).astype(np.float32)
    return wk, bvec


def _prep_x(x_shard):
    # [B_LOC, C, H, W] -> blocked [NBLK, K=(c,hh), B_LOC, W]
    oh0s = np.minimum(np.arange(NBLK) * T, OH - T)            # [18]
    rows = oh0s[:, None, None] + np.arange(HH)[None, None, :]  # [18,1,12]
    rows = np.broadcast_to(rows, (NBLK, C, HH))
    chan = np.broadcast_to(np.arange(C)[None, :, None], (NBLK, C, HH))
    xb = x_shard.transpose(1, 2, 0, 3)[chan.reshape(-1), rows.reshape(-1)]
    return np.ascontiguousarray(xb.reshape(NBLK, K, B_LOC, W))


def _unpack_y(y_s):
    # y_s [NBLK, NPG, 128, SF] -> [B_LOC, OC, OH, OW]
    # dims: (blk, pg, (oc,j), (pair,b2,ow))
    v = y_s.reshape(NBLK, NPG, OC, T, PGP, BPER, OW)
    v = v.transpose(1, 4, 5, 2, 0, 3, 6)      # pg,pair,b2,oc,blk,j,ow
    v = v.reshape(B_LOC, OC, NBLK * T, OW)
    # block 17 sits at oh0 = OH-T = 130; keep only its rows j >= 136-130
    jlo = (NBLK - 1) * T - (OH - T)            # 6
    return np.concatenate([v[:, :, :(NBLK - 1) * T, :],
                           v[:, :, (NBLK - 1) * T + jlo:, :]], axis=2)


def kernel(x, w3, b3, w4, b4, w6, b6):
    nc = _build()
    wk, bvec = _prep_weights(w3, b3, w4, b4, w6, b6)
    x = np.ascontiguousarray(x, dtype=np.float32)
    in_maps = [{"x": _prep_x(x[i * B_LOC:(i + 1) * B_LOC]), "w": wk, "b": bvec}
               for i in range(NCORES)]
    res = run_bass_kernel_spmd(nc, in_maps, list(range(NCORES)))
    out = np.concatenate([_unpack_y(res.results[i]["y"]) for i in range(NCORES)],
                         axis=0)
    return np.ascontiguousarray(out)


# revision 15
# speedup vs baseline: 6.7875x; 1.2347x over previous
import sys
sys.path.insert(0, '/opt/trn_rl_repo')
import numpy as np
import concourse.bass as bass
import concourse.bacc as bacc
import concourse.tile as tile
import concourse.mybir as mybir
from concourse.bass_utils import run_bass_kernel_spmd

C3_TABLE = [(0, 1, 2), (1, 2, 3), (2, 3, 4), (3, 4, 5), (0, 4, 5), (0, 1, 5),
            (0, 1, 2, 3), (1, 2, 3, 4), (2, 3, 4, 5), (0, 3, 4, 5), (0, 1, 4, 5),
            (0, 1, 2, 5), (0, 1, 3, 4), (1, 2, 4, 5), (0, 2, 3, 5),
            (0, 1, 2, 3, 4, 5)]
A = 1.7159
S = 2.0 / 3.0

B, C, H, W = 256, 6, 142, 142
KH = KW = 5
OC = 16
OH, OW = H - 4, W - 4          # 138
NCORES = 8
B_LOC = B // NCORES            # 32

T = 6                          # oh rows per block
HH = T + KH - 1                # 10
NS2 = 2                        # kw taps packed into K (s dim)
K = C * HH * NS2               # 120
M = OC * T                     # 96
NP = 3                         # matmul passes: kw pairs {0,1},{2,3},{4,-}
BPER = 2
NS = BPER * OW                 # 276
NBLK = OH // T                 # 23 exactly
NPAIR = B_LOC // BPER          # 16
PGP = 8
NPG = NPAIR // PGP             # 2
GI = PGP * BPER                # 16
XF = GI * W                    # 2272
SF = PGP * NS                  # 2208

_cache = {}


def _build():
    if 'nc' in _cache:
        return _cache['nc']
    f32 = mybir.dt.float32
    f32r = mybir.dt.float32r
    nc = bacc.Bacc("TRN2", target_bir_lowering=False, debug=False,
                   num_devices=NCORES)
    x_d = nc.dram_tensor("x", [NBLK, K, B_LOC, W], f32, kind="ExternalInput").ap()
    w_d = nc.dram_tensor("w", [K, NP, M], f32, kind="ExternalInput").ap()
    b_d = nc.dram_tensor("b", [M, 1], f32, kind="ExternalInput").ap()
    y_d = nc.dram_tensor("y", [NBLK, NPG, M, SF], f32, kind="ExternalOutput").ap()

    with tile.TileContext(nc) as tc:
        with tc.tile_pool(name="wpool", bufs=1) as wpool, \
             tc.tile_pool(name="xpool", bufs=6) as xpool, \
             tc.tile_pool(name="tpool", bufs=4) as tpool, \
             tc.tile_pool(name="spool", bufs=3) as spool, \
             tc.tile_pool(name="pspool", bufs=1, space="PSUM") as pspool:
            w_sb = wpool.tile([K, NP * M], f32r)
            nc.sync.dma_start(w_sb[:], w_d[:].bitcast(f32r).rearrange("k f m -> k (f m)"))
            b_sb = wpool.tile([M, 1], f32)
            nc.sync.dma_start(b_sb[:], b_d[:])

            for blk in range(NBLK):
                for pg in range(NPG):
                    xt = xpool.tile([K, XF], f32r)
                    src = x_d[blk, :, pg * GI:(pg + 1) * GI, :]
                    src = src.rearrange("k i w -> k (i w)")
                    nc.sync.dma_start(xt[:], src.bitcast(f32r))

                    stage = spool.tile([M, SF], f32)
                    xv = xt[:].rearrange("k (i w) -> k i w", i=GI)
                    pss = [pspool.tile([M, NS], f32, name=f"ps{p_}",
                                       tag=f"ps{p_}") for p_ in range(PGP)]
                    for g in range(NP):
                        for pair in range(PGP):
                            rv = xv[:, pair * BPER:(pair + 1) * BPER, :]
                            nc.tensor.matmul(
                                pss[pair][:],
                                w_sb[:, g * M:(g + 1) * M],
                                rv[:, :, 2 * g:2 * g + OW],
                                start=(g == 0), stop=(g == NP - 1),
                            )
                    for pair in range(PGP):
                        t_sb = tpool.tile([M, NS], f32)
                        nc.scalar.activation(t_sb[:], pss[pair][:],
                                             mybir.ActivationFunctionType.Tanh,
                                             bias=b_sb[:], scale=S)
                        nc.vector.tensor_scalar_mul(
                            stage[:, pair * NS:(pair + 1) * NS], t_sb[:], A)
                    nc.gpsimd.dma_start(y_d[blk, pg], stage[:])
    nc.compile()
    _cache['nc'] = nc
    return nc


def _prep_weights(w3, b3, w4, b4, w6, b6):
    Wd = np.zeros((OC, C, KH, KW), np.float32)
    bias = np.zeros((OC,), np.float32)
    for i, idx in enumerate(C3_TABLE[:6]):
        Wd[i, list(idx)] = w3[i]
        bias[i] = b3[i]
    for i, idx in enumerate(C3_TABLE[6:15]):
        Wd[6 + i, list(idx)] = w4[i]
        bias[6 + i] = b4[i]
    Wd[15, list(C3_TABLE[15])] = w6[0]
    bias[15] = b6[0]

    # K row r = (c*HH + hh)*2 + s ; M col m = oc*T + j ; pass g: kw = 2g+s
    wk = np.zeros((K, NP, M), np.float32)
    for c in range(C):
        for hh in range(HH):
            for j in range(T):
                kh = hh - j
                if not (0 <= kh < KH):
                    continue
                for s in range(NS2):
                    for g in range(NP):
                        kw = 2 * g + s
                        if kw < KW:
                            r = (c * HH + hh) * 2 + s
                            wk[r, g, np.arange(OC) * T + j] = Wd[:, c, kh, kw]
    bvec = (S * bias[np.arange(M) // T]).reshape(M, 1).astype(np.float32)
    return wk, bvec


def _prep_x(x_shard):
    # [B_LOC, C, H, W] -> [NBLK, K=(c,hh,s), B_LOC, W]; s=1 shifted by one col
    xt = x_shard.transpose(1, 2, 0, 3)                  # [C, H, B, W]
    xb = np.zeros((NBLK, C, HH, NS2, B_LOC, W), np.float32)
    rows = (np.arange(NBLK) * T)[:, None] + np.arange(HH)[None, :]  # [23,10]
    g = xt[:, rows]                                     # [C, 23, 10, B, W]
    g = g.transpose(1, 0, 2, 3, 4)                      # [23, C, 10, B, W]
    xb[:, :, :, 0, :, :] = g
    xb[:, :, :, 1, :, :-1] = g[..., 1:]
    return np.ascontiguousarray(xb.reshape(NBLK, K, B_LOC, W))


def _unpack_y(y_s):
    v = y_s.reshape(NBLK, NPG, OC, T, PGP, BPER, OW)
    v = v.transpose(1, 4, 5, 2, 0, 3, 6)                # pg,pair,b2,oc,blk,j,ow
    return v.reshape(B_LOC, OC, OH, OW)


def kernel(x, w3, b3, w4, b4, w6, b6):
    nc = _build()
    w3, b3, w4, b4, w6, b6 = [np.asarray(a, dtype=np.float32)
                              for a in (w3, b3, w4, b4, w6, b6)]
    wk, bvec = _prep_weights(w3, b3, w4, b4, w6, b6)
    x = np.ascontiguousarray(np.asarray(x), dtype=np.float32)
    in_maps = [{"x": _prep_x(x[i * B_LOC:(i + 1) * B_LOC]), "w": wk, "b": bvec}
               for i in range(NCORES)]
    res = run_bass_kernel_spmd(nc, in_maps, list(range(NCORES)))
    out = np.concatenate([_unpack_y(res.results[i]["y"]) for i in range(NCORES)],
                         axis=0)
    return np.ascontiguousarray(out)
